# revision 21
# baseline (speedup 1.0000x reference)
"""Trainium2 Bass kernel for nn_Encoder_Postnet (alignment walk + gather).

Contract: kernel(**inputs) takes the FULL unsharded inputs
(encoder_out [32,512,512] f32, align_phone [32,4096] int, text_phone
[32,512] int) and returns the FULL output [32,4096,512] f32, matching

    vmap(_align_one)(encoder_out, align_phone, text_phone)

where _align_one is a sequential walk producing per-frame encoder-row
indices followed by a row gather.

Strategy: the walk itself is tiny integer work (B*T_ALIGN steps) and is
computed on host; the memory-bound part — materializing the 268 MB
gathered output — runs on 8 NeuronCores, batch-sharded 4 elems/core.

Device programs:
  * fast path (input has the uniform duration-expanded structure, i.e.
    idx[t] == t//d for all t and every frame valid): encoder row r lives
    on partition r % 128 (slot r // 128), so all d replicas of a row
    (output rows t = j*128*d + p*d + k) stay on the SAME partition.
    The host pre-rounds the encoder to bf16 (rel err ~2e-3, well inside
    the 2e-2 gate), halving HBM read traffic. Loads are issued
    just-in-time and split so the first store's 128-row slice lands
    first; the vector engine then replicates rows within-partition into
    an output-ordered exp buffer (one stride-0 broadcast copy per block)
    and the stores stream the output blocks back-to-back over the two
    HWDGE rings at the ~430 GB/s per-core roofline. Since every value
    is already bf16-rounded, the device writes the output as bf16 (half
    the HBM write traffic of f32) and the host upcasts — an exact
    conversion, so the returned f32 array is bit-identical to the f32
    device-output variant (rel err 2.9e-3 vs the f32 reference, inside
    the 2e-2 gate).
  * generic path (any other walk result): indirect-DMA row gather
    (128 rows per descriptor batch) through SBUF, double buffered.
"""

import os

import numpy as np

from concourse import bass, bass_utils, mybir
from concourse.bass_utils import run_bass_kernel_spmd

# If the caller's environment enables tracing (BASS_TRACE=1) but has no
# artifact bucket, don't let the artifact upload kill the run.
_orig_upload = bass_utils.upload_artifacts


def _safe_upload(tmpdir):
    try:
        return _orig_upload(tmpdir)
    except Exception:
        return f"local:{tmpdir}"


bass_utils.upload_artifacts = _safe_upload

B, T_TEXT, T_ALIGN, D = 32, 512, 4096, 512
N_CORES = 8
BPC = B // N_CORES  # batch elems per core
P = 128
J = T_TEXT // P  # free-dim slots per partition for one encoder slice

# "v8" measured best on HW: 57.3 us (bf16 device output + exact host
# upcast) vs 96.0 (v3 f32 stores), 97.1 (v5 broadcast-copy), 99.3
# (v4 SWDGE cast-stores), 102.7 (f32 DVE-expand baseline), 107.4
# (stride-0 replicating stores).
VARIANT = os.environ.get("BASS_VARIANT", "v8")

_PROGRAM_CACHE = {}


# ---------------------------------------------------------------- host scan
def _host_scan(align, text):
    """Replicates the reference jax.lax.scan walk in numpy, vectorized
    over batch. align [B, T_ALIGN] int32, text [B, T_TEXT] int32.
    Returns idx [B, T_ALIGN] int32, valid [B, T_ALIGN] bool."""
    Bn, Ta = align.shape
    Tt = text.shape[1]
    idx = np.zeros((Bn, Ta), np.int32)
    valid = np.ones((Bn, Ta), np.bool_)
    ind = np.zeros(Bn, np.int32)
    before = text[:, 0].copy()
    done = np.zeros(Bn, np.bool_)
    rows = np.arange(Bn)
    for t in range(1, Ta):
        a = align[:, t]
        match = a == before
        inc = np.where(match, ind, ind + 1)
        overflow = inc >= Tt
        new_done = done | (~match & overflow)
        safe = np.minimum(inc, Tt - 1)
        new_before = np.where(match | new_done, before, text[rows, safe])
        new_ind = np.where(new_done, ind, inc)
        idx[:, t] = new_ind
        valid[:, t] = ~new_done
        ind, before, done = new_ind, new_before, new_done
    return idx, valid


def _is_uniform_expansion(align, text):
    """True iff the walk provably yields idx[t] = t // d with every frame
    valid: align is exactly text repeated d times per phone and adjacent
    text phones are distinct (so each run boundary advances by exactly 1
    and the index never overflows)."""
    if T_ALIGN % T_TEXT:
        return False
    d = T_ALIGN // T_TEXT
    if not np.array_equal(align, np.repeat(text, d, axis=1)):
        return False
    return bool(np.all(text[:, 1:] != text[:, :-1]))


# ------------------------------------------------------------- fast program
def _build_fast(d, variant):
    """Per-core program: enc (bf16) [BPC,T_TEXT,D] -> out (f32)
    [BPC,T_ALIGN,D] where out[b, t] = f32(enc[b, t // d]).

    Loads (HWDGE, plain bf16) are split so the first store's slice
    arrives first: sync ring carries (b0,j0); scalar ring carries
    (b0,j1..3), (b1), (b2), (b3). The vector engine replicates each
    loaded [P, D] tile d times into an output-ordered exp buffer
    (4-deep, per-buffer semaphores); every store is then one fully
    contiguous 2 MB block (16 KB per partition line).

    v3: DVE copy upcasts bf16 -> f32 (d separate row copies, 4-deep exp);
        stores alternate the two HWDGE rings (sync/scalar).
    v4: exp stays bf16 (halves DVE + DMA SBUF read traffic); all stores
        are SWDGE cast-DMAs (bf16 -> f32) on gpsimd.
    v5: like v3, but each block is ONE broadcast tensor_copy (source
        stride 0 over the replica axis — halves DVE latency and cuts
        instruction count 8x) and exp is 8-deep so the DVE runs ~8
        blocks ahead of the stores."""
    nc = bass.Bass()
    f32 = mybir.dt.float32
    bf16 = mybir.dt.bfloat16
    exp_dt = bf16 if variant in ("v4", "v8") else f32
    enc = nc.dram_tensor("enc", [BPC, T_TEXT, D], bf16, kind="ExternalInput")
    # v8: the values are already bf16-rounded (host pre-cast); writing the
    # output as bf16 and upcasting on host returns the identical f32 array
    # while halving the dominant HBM write traffic.
    out_dt = bf16 if variant == "v8" else f32
    out = nc.dram_tensor("out", [BPC, T_ALIGN, D], out_dt, kind="ExternalOutput")
    FREE = BPC * J * D
    NBUF = 8 if variant in ("v5", "v8") else 4
    NBLK = BPC * J
    FREE_EXP = NBUF * d * D
    NRND = NBLK // NBUF

    with (
        nc.sbuf_tensor("enc_sb", [P, BPC, J, D], bf16) as enc_sb,
        nc.sbuf_tensor("exp", [P, NBUF, d * D], exp_dt) as exp,
        nc.sbuf_tensor("exp_b", [P, 2, d * D], bf16) as exp_b_t,
        nc.semaphore("la") as la,
        nc.semaphore("lb") as lb,
        nc.semaphore("l1") as l1,
        nc.semaphore("l2") as l2,
        nc.semaphore("l3") as l3,
        nc.Block() as block,
    ):
        csem = [nc.alloc_semaphore(f"c{b}") for b in range(NBUF)]
        wsem = [nc.alloc_semaphore(f"w{b}") for b in range(NBUF)]
        # load sem for block i = b*J + j: (b0,j0)->la, (b0,j>0)->lb,
        # (b1)->l1, (b2)->l2, (b3)->l3
        lsem = [la, lb, lb, lb, l1, l1, l1, l1, l2, l2, l2, l2, l3, l3, l3, l3]

        def load(eng, b, j0, nj, sem):
            eng.dma_start(
                bass.AP(enc_sb, (b * J + j0) * D, [[FREE, P], [D, nj], [1, D]]),
                bass.AP(
                    enc,
                    (b * T_TEXT + j0 * P) * D,
                    [[D, P], [P * D, nj], [1, D]],
                ),
            ).then_inc(sem, 16)

        copies_per_block = 1 if variant in ("v5", "v8") else d

        def store(eng, i):
            b, j = divmod(i, J)
            buf, rnd = i % NBUF, i // NBUF
            eng.wait_ge(csem[buf], (rnd + 1) * copies_per_block)
            eng.dma_start(
                bass.AP(
                    out,
                    (b * T_ALIGN + j * P * d) * D,
                    [[d * D, P], [1, d * D]],
                ),
                bass.AP(exp, buf * d * D, [[FREE_EXP, P], [1, d * D]]),
            ).then_inc(wsem[buf], 16)

        def store_part(eng, i, k0, nk, cwait):
            """Store replicas [k0, k0+nk) of block i as one DMA (the nk*D
            elems are contiguous within each partition line). cwait is the
            csem[buf] threshold to wait for (None = no wait, same-engine
            ordering already guarantees the data)."""
            b, j = divmod(i, J)
            buf = i % NBUF
            if cwait is not None:
                eng.wait_ge(csem[buf], cwait)
            eng.dma_start(
                bass.AP(
                    out,
                    (b * T_ALIGN + j * P * d + k0) * D,
                    [[d * D, P], [1, nk * D]],
                ),
                bass.AP(exp, (buf * d + k0) * D, [[FREE_EXP, P], [1, nk * D]]),
            ).then_inc(wsem[buf], 16)

        if variant == "v6":
            # Stores of some blocks are split into sub-stores for a faster
            # ramp (first block per ring) and a shorter drain tail (last
            # block per ring); each sub-store incs wsem[buf] by 16.
            STORE_INCS = {i: 16 for i in range(NBLK)}
            STORE_INCS[0] = STORE_INCS[1] = STORE_INCS[NBLK - 2] = 32
            STORE_INCS[NBLK - 1] = 64

            def wsem_before(i):
                # total wsem[buf] incs from earlier blocks on the same buffer
                return sum(STORE_INCS[k] for k in range(i % NBUF, i, NBUF))

            def dve_block(vector, i):
                b, j = divmod(i, J)
                buf, rnd = i % NBUF, i // NBUF
                vector.wait_ge(lsem[i], 16)
                if rnd:
                    vector.wait_ge(wsem[buf], wsem_before(i))
                for k in range(d):
                    vector.tensor_copy(
                        bass.AP(exp, (buf * d + k) * D, [[FREE_EXP, P], [1, D]]),
                        bass.AP(enc_sb, (b * J + j) * D, [[FREE, P], [1, D]]),
                    ).then_inc(csem[buf], 1)

            @block.vector
            def _(vector):
                for i in range(NBLK):
                    dve_block(vector, i)

            @block.gpsimd
            def _(gpsimd):
                load(gpsimd, 1, 0, J, l1)
                load(gpsimd, 2, 0, J, l2)
                load(gpsimd, 3, 0, J, l3)

            @block.scalar
            def _(scalar):
                load(scalar, 0, 1, J - 1, lb)
                # first store on this ring launches after 4 of 8 replicas
                store_part(scalar, 1, 0, d // 2, d // 2)
                store_part(scalar, 1, d // 2, d // 2, d)
                for i in range(3, NBLK - 1, 2):
                    store(scalar, i)
                # last block: 4 sub-stores so the final receipt is short
                q = d // 4
                for k2 in range(4):
                    store_part(
                        scalar, NBLK - 1, k2 * q, q, 3 * d + (k2 + 1) * q
                    )

            @block.sync
            def _(sync):
                load(sync, 0, 0, 1, la)
                store_part(sync, 0, 0, d // 2, d // 2)
                store_part(sync, 0, d // 2, d // 2, d)
                for i in range(2, NBLK - 2, 2):
                    store(sync, i)
                store_part(sync, NBLK - 2, 0, d // 2, 3 * d + d // 2)
                store_part(sync, NBLK - 2, d // 2, d // 2, 4 * d)
                for buf in range(NBUF):
                    sync.wait_ge(wsem[buf], wsem_before(buf + NBLK))

            return nc

        if variant == "v7":
            # Mixed stores: fabric relief. f32 HWDGE stores read SBUF 1:1
            # with their HBM writes, capping steady state at the ~435 GB/s
            # SBUF AXI fabric. Routing every third block through a SWDGE
            # cast-store (bf16 exp -> f32 out, gpsimd) drops fabric load to
            # ~84% of HBM write load so the HBM side (~460 GB/s) binds.
            BF_BLOCKS = [i for i in range(NBLK) if i % 3 == 2]  # 2,5,8,11,14
            F32_BLOCKS = [i for i in range(NBLK) if i % 3 != 2]
            NBUF_B = 2
            exp_b = exp_b_t
            FREE_EXPB = NBUF_B * d * D
            cb = [nc.alloc_semaphore(f"cb{b}") for b in range(NBUF_B)]
            wb = [nc.alloc_semaphore(f"wb{b}") for b in range(NBUF_B)]

            def pool_of(i):
                if i in BF_BLOCKS:
                    s = BF_BLOCKS.index(i)
                    return cb, wb, s % NBUF_B, s // NBUF_B
                s = F32_BLOCKS.index(i)
                return csem, wsem, s % NBUF, s // NBUF

            def store7(eng, i):
                b, j = divmod(i, J)
                cs, ws, buf, rnd = pool_of(i)
                eng.wait_ge(cs[buf], (rnd + 1) * d)
                if i in BF_BLOCKS:
                    src = bass.AP(exp_b, buf * d * D, [[FREE_EXPB, P], [1, d * D]])
                else:
                    src = bass.AP(exp, buf * d * D, [[FREE_EXP, P], [1, d * D]])
                eng.dma_start(
                    bass.AP(
                        out,
                        (b * T_ALIGN + j * P * d) * D,
                        [[d * D, P], [1, d * D]],
                    ),
                    src,
                ).then_inc(ws[buf], 16)

            @block.vector
            def _(vector):
                for i in range(NBLK):
                    b, j = divmod(i, J)
                    cs, ws, buf, rnd = pool_of(i)
                    vector.wait_ge(lsem[i], 16)
                    if rnd:
                        vector.wait_ge(ws[buf], rnd * 16)
                    for k in range(d):
                        if i in BF_BLOCKS:
                            dst = bass.AP(
                                exp_b, (buf * d + k) * D, [[FREE_EXPB, P], [1, D]]
                            )
                        else:
                            dst = bass.AP(
                                exp, (buf * d + k) * D, [[FREE_EXP, P], [1, D]]
                            )
                        vector.tensor_copy(
                            dst,
                            bass.AP(enc_sb, (b * J + j) * D, [[FREE, P], [1, D]]),
                        ).then_inc(cs[buf], 1)

            @block.gpsimd
            def _(gpsimd):
                load(gpsimd, 1, 0, J, l1)
                load(gpsimd, 2, 0, J, l2)
                load(gpsimd, 3, 0, J, l3)
                for i in BF_BLOCKS:
                    store7(gpsimd, i)

            @block.scalar
            def _(scalar):
                load(scalar, 0, 1, J - 1, lb)
                for i in F32_BLOCKS:
                    if i % 3 == 1:
                        store7(scalar, i)

            @block.sync
            def _(sync):
                load(sync, 0, 0, 1, la)
                for i in F32_BLOCKS:
                    if i % 3 == 0:
                        store7(sync, i)
                nf = [len([x for x in F32_BLOCKS if F32_BLOCKS.index(x) % NBUF == b]) for b in range(NBUF)]
                for buf in range(NBUF):
                    sync.wait_ge(wsem[buf], nf[buf] * 16)
                nb = [len([x for x in BF_BLOCKS if BF_BLOCKS.index(x) % NBUF_B == b]) for b in range(NBUF_B)]
                for buf in range(NBUF_B):
                    sync.wait_ge(wb[buf], nb[buf] * 16)

            return nc

        @block.vector
        def _(vector):
            for i in range(NBLK):
                b, j = divmod(i, J)
                buf, rnd = i % NBUF, i // NBUF
                vector.wait_ge(lsem[i], 16)
                if rnd:
                    # the store that read this buffer must be done
                    vector.wait_ge(wsem[buf], rnd * 16)
                if variant in ("v5", "v8"):
                    # single broadcast copy: replicate the [P, D] tile d
                    # times along the middle axis (source stride 0)
                    vector.tensor_copy(
                        bass.AP(exp, buf * d * D, [[FREE_EXP, P], [D, d], [1, D]]),
                        bass.AP(enc_sb, (b * J + j) * D, [[FREE, P], [0, d], [1, D]]),
                    ).then_inc(csem[buf], 1)
                else:
                    for k in range(d):
                        vector.tensor_copy(
                            bass.AP(exp, (buf * d + k) * D, [[FREE_EXP, P], [1, D]]),
                            bass.AP(enc_sb, (b * J + j) * D, [[FREE, P], [1, D]]),
                        ).then_inc(csem[buf], 1)

        if variant in ("v3", "v5", "v8"):

            @block.scalar
            def _(scalar):
                load(scalar, 0, 1, J - 1, lb)
                load(scalar, 1, 0, J, l1)
                load(scalar, 2, 0, J, l2)
                load(scalar, 3, 0, J, l3)
                for i in range(1, NBLK, 2):
                    store(scalar, i)

            @block.sync
            def _(sync):
                load(sync, 0, 0, 1, la)
                for i in range(0, NBLK, 2):
                    store(sync, i)
                for buf in range(NBUF):
                    sync.wait_ge(wsem[buf], NRND * 16)

        else:  # v4

            @block.scalar
            def _(scalar):
                load(scalar, 0, 1, J - 1, lb)
                load(scalar, 1, 0, J, l1)
                load(scalar, 2, 0, J, l2)
                load(scalar, 3, 0, J, l3)

            @block.sync
            def _(sync):
                load(sync, 0, 0, 1, la)

            @block.gpsimd
            def _(gpsimd):
                for i in range(NBLK):
                    store(gpsimd, i)
                for buf in range(NBUF):
                    gpsimd.wait_ge(wsem[buf], NRND * 16)

    return nc


# ---------------------------------------------------------- generic program
def _build_generic():
    """Per-core program: flat encoder table enc [BPC*(T_TEXT+1), D] (one
    zero row appended per batch elem), per-frame row indices idx
    [P, BPC*NCH] (host pre-transposed, already offset into the flat
    table, invalid frames pointed at the zero row) -> out [BPC,T_ALIGN,D].
    Gathers 128 rows per indirect DMA, double buffered through SBUF."""
    nc = bass.Bass()
    f32 = mybir.dt.float32
    i32 = mybir.dt.int32
    NCH = T_ALIGN // P  # index chunks per batch elem
    enc = nc.dram_tensor("enc", [BPC * (T_TEXT + 1), D], f32, kind="ExternalInput")
    idxt = nc.dram_tensor("idx", [P, BPC * NCH], i32, kind="ExternalInput")
    out = nc.dram_tensor("out", [BPC, T_ALIGN, D], f32, kind="ExternalOutput")
    NBUF = 4

    with (
        nc.sbuf_tensor("idx_sb", [P, BPC * NCH], i32) as idx_sb,
        nc.sbuf_tensor("row_sb", [P, NBUF, D], f32) as row_sb,
        nc.semaphore("isem") as isem,
        nc.semaphore("g0") as g0,
        nc.semaphore("g1") as g1,
        nc.semaphore("g2") as g2,
        nc.semaphore("g3") as g3,
        nc.semaphore("w0") as w0,
        nc.semaphore("w1") as w1,
        nc.semaphore("w2") as w2,
        nc.semaphore("w3") as w3,
        nc.Block() as block,
    ):
        # Per-buffer semaphores: concurrent DMAs complete out of order, so
        # a shared counter with (i+1)*16 thresholds would let a later
        # gather's completion unblock an earlier chunk's store. With one
        # sem per buffer, each threshold only counts the DMA chain of its
        # own buffer.
        gsem = [g0, g1, g2, g3]
        wsem = [w0, w1, w2, w3]

        @block.gpsimd
        def _(gpsimd):
            gpsimd.dma_start(idx_sb[:], idxt[:]).then_inc(isem, 16)
            gpsimd.wait_ge(isem, 16)
            for i in range(BPC * NCH):
                buf = i % NBUF
                rnd = i // NBUF
                if i >= NBUF:
                    # the store that reads this buffer must be done
                    gpsimd.wait_ge(wsem[buf], rnd * 16)
                gpsimd.indirect_dma_start(
                    out=row_sb[:, buf, :],
                    out_offset=None,
                    in_=enc[:],
                    in_offset=bass.IndirectOffsetOnAxis(
                        ap=idx_sb[:, i : i + 1], axis=0
                    ),
                ).then_inc(gsem[buf], 16)

        @block.sync
        def _(sync):
            for i in range(BPC * NCH):
                b, c = divmod(i, NCH)
                buf = i % NBUF
                rnd = i // NBUF
                sync.wait_ge(gsem[buf], (rnd + 1) * 16)
                sync.dma_start(
                    out[b, c * P : (c + 1) * P, :], row_sb[:, buf, :]
                ).then_inc(wsem[buf], 16)
            for buf in range(NBUF):
                sync.wait_ge(wsem[buf], (BPC * NCH // NBUF) * 16)

    return nc


# ------------------------------------------------------------------- driver
def _run(encoder_out, align_phone, text_phone, trace=False):
    enc = np.ascontiguousarray(np.asarray(encoder_out, dtype=np.float32))
    align = np.asarray(align_phone).astype(np.int32)
    text = np.asarray(text_phone).astype(np.int32)
    assert enc.shape == (B, T_TEXT, D), enc.shape
    assert align.shape == (B, T_ALIGN), align.shape
    assert text.shape == (B, T_TEXT), text.shape

    if _is_uniform_expansion(align, text):
        d = T_ALIGN // T_TEXT
        key = ("fast3", d, VARIANT)
        if key not in _PROGRAM_CACHE:
            _PROGRAM_CACHE[key] = _build_fast(d, VARIANT)
        nc = _PROGRAM_CACHE[key]
        import ml_dtypes

        enc_in = enc.astype(ml_dtypes.bfloat16)
        in_maps = [
            {"enc": np.ascontiguousarray(enc_in[i * BPC : (i + 1) * BPC])}
            for i in range(N_CORES)
        ]
    else:
        idx, valid = _host_scan(align, text)
        # route invalid frames at the per-batch zero pad row, flatten the
        # batch into the row index so the indirect table offset is 0
        idx_safe = np.where(valid, idx, T_TEXT).astype(np.int32)
        idx_safe += (np.arange(B, dtype=np.int32) * (T_TEXT + 1))[:, None]
        enc_pad = np.concatenate(
            [enc, np.zeros((B, 1, D), np.float32)], axis=1
        )  # [B, T_TEXT+1, D]
        NCH = T_ALIGN // P
        if "generic" not in _PROGRAM_CACHE:
            _PROGRAM_CACHE["generic"] = _build_generic()
        nc = _PROGRAM_CACHE["generic"]
        in_maps = []
        for i in range(N_CORES):
            sl = slice(i * BPC, (i + 1) * BPC)
            enc_i = enc_pad[sl].reshape(BPC * (T_TEXT + 1), D)
            # idx for this core, rebased to core-local flat rows, laid out
            # [P, BPC*NCH] so partition p holds row index for frame c*P+p
            idx_i = idx_safe[sl] - i * BPC * (T_TEXT + 1)
            idx_i = np.ascontiguousarray(
                idx_i.reshape(BPC, NCH, P).transpose(2, 0, 1).reshape(P, BPC * NCH)
            )
            in_maps.append(
                {"enc": np.ascontiguousarray(enc_i), "idx": idx_i}
            )

    res = run_bass_kernel_spmd(nc, in_maps, list(range(N_CORES)), trace=trace)
    full = np.concatenate([r["out"] for r in res.results], axis=0)
    if full.dtype != np.float32:
        # v8 writes the (already bf16-rounded) output as bf16 on device;
        # the upcast back to f32 is exact
        full = full.astype(np.float32)
    return full, res


def kernel(encoder_out, align_phone, text_phone):
    full, _ = _run(encoder_out, align_phone, text_phone)
    return full
